# revision 51
# baseline (speedup 1.0000x reference)
"""Multi-head attention (QKV projection + masked softmax + PV) on 8 TRN2
NeuronCores.

Sharding: data-parallel over batch (B=2 -> 2 groups of 4 cores), tensor
parallel over heads (16 heads -> 4 heads per core). Each core computes full
F x T attention for its 4 heads.

Per-core device algorithm (kept transposed so the softmax reduction lands on
the TensorE contraction dim; all matmuls bf16, PSUM fp32):
  Q^T[h,f] = wq^T @ from^T        K^T[h,t] = wk^T @ to^T
  V[t,hh]  = to^T^T @ wv   (+ a ones column per head for the softmax sums)
  S^T[t,f] = K^T(stationary, zero-padded to K=128) x Q^T(moving)
  E = exp(S^T/8) (ScalarE, PSUM->SBUF, bf16);  E *= mask^T (bf16)
  ctx^T[h,f] (+ sums[f] via the ones column) = sum_t V x E
  out = ctx^T * (1/sums)  (recip on a [16,128] gather so it uses 16 DVE
                           lanes; selector matmuls broadcast it down the
                           h-partitions per 128-col f-chunk)

Pipeline notes: j0 runs with PSUM = two 2-bank S regions + 1 ctx bank +
3 rotating utility banks so the K/V/QT emissions (all interleaved into j0's
attention stream by deadline) never serialize on a psum drain; j1-3 swap the
pools to two 3-bank S regions (groups of 3,3,3,3,2,2 t-tiles -> fewer,
bigger exps) + 1 utility bank.  Per-j softmax normalization is deferred into
the next j: sums are DMA-gathered into a [16,128] tile (16-lane reciprocal,
f16 out), K=16 selector matmuls broadcast the recip down the h-partitions
per 128-col f-chunk.  The 2MB mask prefetch for j+1 is gated behind h1 so it
doesn't steal DMA bandwidth from the gathers.  Host prepacks every bulk
tensor partition-major so each load is one large contiguous DMA.

Host does the cheap layout work: pre-transposes from/to/mask (bf16), slices
weights per head group, transposes the [4,64,2048] bf16 per-core results back
into [B,F,N,H] fp32.
"""

import os
import sys

for _p in ("/opt/trn_rl_repo",):
    if os.path.isdir(_p) and _p not in sys.path:
        sys.path.insert(0, _p)

import numpy as np
import ml_dtypes

import concourse.tile as tile
from concourse import bacc, mybir
from concourse.bass_utils import run_bass_kernel_spmd

B, F, T, D, N, H = 2, 2048, 2048, 1024, 16, 64
NCORES = 8
HPC = N // (NCORES // B)  # heads per core = 4
NG = HPC // 2             # 128-partition head groups (2 heads each) = 2
FB = 512                  # f-block (psum bank width in fp32)
NJ = F // FB              # 4
NT = T // 128             # 16 t-tiles
NK = D // 128             # 8 contraction tiles
HP1 = H + 1               # head V columns incl. the ones column
W = 2                     # t-tiles per S/exp group
NGRP = NT // W            # 8 groups

F32 = mybir.dt.float32
F16 = mybir.dt.float16
BF16 = mybir.dt.bfloat16


def _emit_k_piece(nc, ps_util, kv, sbuf, tb, g):
    """K^T for t-block tb (FB cols), head-group g: parity-split into KTe/KTo."""
    (QT, KTe, KTo, Vsb, bias_sb, bv_sb, vones_sb) = sbuf
    toT_sb, wk_sb = kv["toT_sb"], kv["wk_sb"]
    ps_qk = ps_util.tile([128, FB], F32, tag="util", name="ps_k")
    for k in range(NK):
        nc.tensor.matmul(
            ps_qk[:],
            wk_sb[:, k, g * 128:(g + 1) * 128],
            toT_sb[:, tb, k, :],
            start=(k == 0),
            stop=(k == NK - 1),
        )
    nc.vector.tensor_scalar_add(
        KTe[0:64, g, tb * FB:(tb + 1) * FB],
        ps_qk[0:64, :],
        bias_sb[0:64, NG + g:NG + g + 1],
    )
    nc.vector.tensor_scalar_add(
        KTo[64:128, g, tb * FB:(tb + 1) * FB],
        ps_qk[64:128, :],
        bias_sb[64:128, NG + g:NG + g + 1],
    )


def _emit_v_piece(nc, ps_util, kv, sbuf, ti):
    (QT, KTe, KTo, Vsb, bias_sb, bv_sb, vones_sb) = sbuf
    toT_sb, wv_sb = kv["toT_sb"], kv["wv_sb"]
    ps_v = ps_util.tile([128, HPC * H], F32, tag="util", name="ps_v")
    tb, o = divmod(ti, 4)
    for k in range(NK):
        nc.tensor.matmul(
            ps_v[:],
            toT_sb[:, tb, k, o * 128:(o + 1) * 128],
            wv_sb[:, k, :],
            start=(k == 0),
            stop=False,
        )
    nc.tensor.matmul(ps_v[:], vones_sb[:], bv_sb[:], start=False, stop=True)
    # one strided copy: psum [128,(nl h)] -> Vsb columns nl*65..nl*65+63
    nc.vector.tensor_copy(
        Vsb[:, ti, 0:HPC * HP1].rearrange("p (nl c) -> p nl c", nl=HPC)[:, :, 0:H],
        ps_v[:].rearrange("p (nl c) -> p nl c", nl=HPC),
    )


def _emit_qt_piece(nc, ps_util, wq_sb, fromT_sb, QT, bias_sb, j, g):
    ps_q = ps_util.tile([128, FB], F32, tag="util", name="ps_q")
    for k in range(NK):
        nc.tensor.matmul(
            ps_q[:],
            wq_sb[:, k, g * 128:(g + 1) * 128],
            fromT_sb[:, j, k, :],
            start=(k == 0),
            stop=(k == NK - 1),
        )
    nc.vector.tensor_scalar_add(
        QT[:, g, j * FB:(j + 1) * FB],
        ps_q[:],
        bias_sb[:, g:g + 1],
    )


def _program():
    nc = bacc.Bacc(None, target_bir_lowering=False)
    # all bulk tensors arrive host-prepacked partition-major & contiguous
    fromTp = nc.declare_dram_parameter("fromTp", [128, NJ, NK, FB], BF16, isOutput=False)
    toTp = nc.declare_dram_parameter("toTp", [128, NJ, NK, FB], BF16, isOutput=False)
    maskp = nc.declare_dram_parameter("maskp", [128, NJ, NT, FB], BF16, isOutput=False)
    wq = nc.declare_dram_parameter("wq", [128, NK, HPC * H], BF16, isOutput=False)
    wk = nc.declare_dram_parameter("wk", [128, NK, HPC * H], BF16, isOutput=False)
    wv = nc.declare_dram_parameter("wv", [128, NK, HPC * H], BF16, isOutput=False)
    bqk = nc.declare_dram_parameter("bqk", [128, 2 * NG], F32, isOutput=False)
    # bv padded to K=128 (row 0 = bv, rest zero) for a mode-switch-free matmul
    bv_pad = nc.declare_dram_parameter("bv_pad", [128, HPC * H], BF16, isOutput=False)
    # all-ones row 0 (rest zero): stationary operand of the bv matmul
    vones = nc.declare_dram_parameter("vones", [128, 128], BF16, isOutput=False)
    # selector blocks: ones_bc[p, 4*nn+c, m] = (p == 4*nn+c): broadcasts the
    # recip16 row for head nn, f-chunk c down all 128 output partitions
    ones_bc = nc.declare_dram_parameter("ones_bc", [16, 16, 128], F16, isOutput=False)
    out_ctx = nc.declare_dram_parameter("out_ctx", [HPC, H, F], BF16, isOutput=True)

    with tile.TileContext(nc) as tc:
        with (
            tc.tile_pool(name="persist", bufs=1) as persist,
            tc.tile_pool(name="pmask", bufs=2) as pmask,
            tc.tile_pool(name="pexp", bufs=4) as pexp,
            tc.tile_pool(name="pctx", bufs=8) as pctx,
            tc.tile_pool(name="pout", bufs=3) as pout,
            tc.tile_pool(name="pnorm", bufs=2) as pnorm,
            tc.tile_pool(name="ps_c", bufs=1, space="PSUM") as ps_c,
        ):
            # j0 PSUM: 2x2-bank S regions + 3 utility banks (emission
            # pipelining); j1-3 swap to 2x3-bank S regions (bigger fused
            # exps) + 1 utility bank (only the norm broadcasts need it)
            ps_s_cm = tc.tile_pool(name="ps_s", bufs=2, space="PSUM")
            ps_s = ps_s_cm.__enter__()
            ps_u_cm = tc.tile_pool(name="ps_u", bufs=3, space="PSUM")
            ps_util = ps_u_cm.__enter__()
            QT = persist.tile([128, NG, F], BF16)        # [h-in-group, g, f]
            # K^T per head parity, dead half zeroed so S can contract K=128
            KTe = persist.tile([128, NG, T], BF16)       # heads 2g   in rows 0-63
            KTo = persist.tile([128, NG, T], BF16)       # heads 2g+1 in rows 64-127
            Vsb = persist.tile([128, NT, HPC * HP1], BF16)
            bias_sb = persist.tile([128, 2 * NG], F32)
            bv_sb = persist.tile([128, HPC * H], BF16)
            vones_sb = persist.tile([128, 128], BF16)
            ones_bc_sb = persist.tile([16, 16, 128], F16)
            toT_sb = persist.tile([128, NJ, NK, FB], BF16)
            fromT_sb = persist.tile([128, NJ, NK, FB], BF16)
            wq_sb = persist.tile([128, NK, HPC * H], BF16)
            wk_sb = persist.tile([128, NK, HPC * H], BF16)
            wv_sb = persist.tile([128, NK, HPC * H], BF16)

            masks = {}
            masks[0] = pmask.tile([128, NT, FB], BF16, tag="mask", name="mask")

            # critical path first on the sync HWDGE ring (contiguous loads)
            nc.sync.dma_start(wk_sb[:, 0:2], wk[:, 0:2])
            nc.sync.dma_start(toT_sb[:, 0, 0:2], toTp[:, 0, 0:2])
            nc.sync.dma_start(wk_sb[:, 2:NK], wk[:, 2:NK])
            nc.sync.dma_start(toT_sb[:, 0, 2:5], toTp[:, 0, 2:5])
            nc.sync.dma_start(toT_sb[:, 0, 5:NK], toTp[:, 0, 5:NK])
            # small constants on the gpsimd (SWDGE) ring, off the bulk path
            nc.gpsimd.dma_start(bias_sb[:], bqk[:])
            nc.gpsimd.dma_start(bv_sb[:], bv_pad[:])
            nc.gpsimd.dma_start(vones_sb[:], vones[:])
            nc.gpsimd.dma_start(ones_bc_sb[:], ones_bc[:])
            # load the exp table before the attention stream needs it
            act_warm = persist.tile([1, 1], F32)
            nc.scalar.activation(act_warm[:], bias_sb[0:1, 0:1],
                                 mybir.ActivationFunctionType.Exp)
            nc.vector.memset(KTe[64:128, :, :], 0.0)
            nc.gpsimd.memset(KTo[0:64, :, :], 0.0)

            for nl in range(HPC):
                nc.vector.memset(Vsb[:, :, nl * HP1 + H], 1.0)

            # remaining bulk loads in first-use order
            nc.sync.dma_start(wq_sb[:], wq[:])
            nc.sync.dma_start(fromT_sb[:, 0, 0:4], fromTp[:, 0, 0:4])
            nc.sync.dma_start(fromT_sb[:, 0, 4:NK], fromTp[:, 0, 4:NK])
            nc.sync.dma_start(masks[0][:, 0:2, :], maskp[:, 0, 0:2, :])
            nc.sync.dma_start(toT_sb[:, 1], toTp[:, 1])
            nc.sync.dma_start(masks[0][:, 2:8, :], maskp[:, 0, 2:8, :])
            nc.sync.dma_start(wv_sb[:], wv[:])
            nc.sync.dma_start(toT_sb[:, 2], toTp[:, 2])
            nc.sync.dma_start(masks[0][:, 8:NT, :], maskp[:, 0, 8:NT, :])
            nc.sync.dma_start(toT_sb[:, 3], toTp[:, 3])
            for jb in range(1, NJ):
                nc.sync.dma_start(fromT_sb[:, jb], fromTp[:, jb])

            sbuf_t = (QT, KTe, KTo, Vsb, bias_sb, bv_sb, vones_sb)
            kv = dict(toT_sb=toT_sb, wk_sb=wk_sb, wv_sb=wv_sb)

            # ---- prefix: just enough K/Q for (j0, head0, group0) ----
            for g in range(NG):
                _emit_k_piece(nc, ps_util, kv, sbuf_t, 0, g)
            _emit_qt_piece(nc, ps_util, wq_sb, fromT_sb, QT, bias_sb, 0, 0)

            # interleave slots: emissions issued just before group g of
            # (j, head n).  j0/head0 carries the K and V deadlines; head3 of
            # every j emits QT for j+1.
            def slots(j, n, g):
                out = []
                if j == 0 and n == 0:
                    if g == 1:
                        out += [("v", 0), ("v", 1)]
                    elif g == 2:
                        out += [("k", 1, 0), ("k", 1, 1), ("v", 2), ("v", 3)]
                    elif g == 3:
                        out += [("v", 4), ("v", 5), ("q", 0, 1)]
                    elif g == 4:
                        out += [("k", 2, 0), ("k", 2, 1), ("v", 6), ("v", 7)]
                    elif g == 5:
                        out += [("v", 8), ("v", 9)]
                    elif g == 6:
                        out += [("k", 3, 0), ("k", 3, 1), ("v", 10), ("v", 11)]
                    elif g == 7:
                        out += [("v", 12), ("v", 13), ("v", 14), ("v", 15)]
                if j == 0 and n == 2:
                    if g == 2:
                        out += [("q", 1, 0)]
                    elif g == 5:
                        out += [("q", 1, 1)]
                # QT for j+2 rides j+1's PE slack (j1-3 are exp-paced)
                if 0 < j < NJ - 1 and n in (1, 2) and g == 2:
                    out += [("q", j + 1, 0 if n == 1 else 1)]
                return out

            prev_norm = [None]   # (sums16, recip16h, ctx_keep, j-1) deferral
            for j in range(NJ):
                if j == 1:
                    ps_u_cm.__exit__(None, None, None)
                    ps_s_cm.__exit__(None, None, None)
                    ps_s_cm = tc.tile_pool(name="ps_s3", bufs=2, space="PSUM")
                    ps_s = ps_s_cm.__enter__()
                    ps_u_cm = tc.tile_pool(name="ps_u1", bufs=1, space="PSUM")
                    ps_util = ps_u_cm.__enter__()
                grps = [2] * 8 if j == 0 else [3, 3, 3, 3, 2, 2]
                gt0 = [sum(grps[:i]) for i in range(len(grps))]
                exp_tag = "exp2" if j == 0 else "exp3"
                mask_j = masks.pop(j)
                mask_next_h = [None]
                if j + 1 < NJ:
                    mask_next_h[0] = pmask.tile([128, NT, FB], BF16,
                                                tag="mask", name="mask")
                    masks[j + 1] = mask_next_h[0]
                # sums gathered as one [16,128] tile: row 4n+c = head n,
                # f-chunk c; one 16-lane reciprocal per j covers all heads.
                # Rows default to 1.0 so a final-j early recip of not-yet-
                # gathered rows stays finite.
                final = j == NJ - 1
                sums16 = pnorm.tile([16, 128], F32, tag="sums", name="sums16")
                recip16h = pnorm.tile([16, 128], F16, tag="recip", name="recip16h")
                sums4 = recip4 = None
                if final:
                    nc.vector.memset(sums16[:], 1.0)
                    sums4 = pnorm.tile([4, 128], F32, tag="sums4", name="sums4")
                    recip4 = pnorm.tile([4, 128], F16, tag="recip4", name="recip4")
                ctx_keep = []

                def _bc_out(nn, ctx_sb, rh, j=j):
                    small = rh.partition_size() == 4
                    ps_bc = ps_util.tile([128, FB], F32, tag="util", name="ps_bc")
                    for c in range(4):
                        sel = (ones_bc_sb[0:4, c, :] if small
                               else ones_bc_sb[:, 4 * nn + c, :])
                        nc.tensor.matmul(
                            ps_bc[:, c * 128:(c + 1) * 128],
                            sel,
                            rh[:],
                            start=True, stop=True,
                        )
                    out_sb = pout.tile([H, FB], BF16, tag="out")
                    nc.vector.tensor_mul(out_sb[:], ctx_sb[0:H, :], ps_bc[0:H, :])
                    eng = nc.sync if j == NJ - 1 else nc.gpsimd
                    eng.dma_start(out_ctx[nn, :, j * FB:(j + 1) * FB], out_sb[:])

                def _emit_pv(pend):
                    """PV matmuls for a lagged exp group (may belong to the
                    previous head); finishes the head when its t15 lands."""
                    pn, p_ps, pt0, pw, pex = pend
                    for i in range(pw):
                        ti = pt0 + i
                        nc.tensor.matmul(
                            p_ps[:],
                            Vsb[:, ti, pn * HP1:(pn + 1) * HP1],
                            pex[:, i, :],
                            start=(ti == 0), stop=(ti == NT - 1),
                        )
                    if pt0 + pw < NT:
                        return
                    ctx_sb = pctx.tile([HP1, FB], F32, tag="ctx_sb",
                                       name="ctx_sb")
                    nc.vector.tensor_copy(ctx_sb[:], p_ps[:])
                    # this head's sums row, one chunk per sums partition
                    # (final head: split across two rings to halve latency)
                    sdst = sums4 if (final and pn == 3) else None
                    for c in range(4):
                        eng = (nc.sync if (pn == 3 and c % 2)
                               else nc.gpsimd)
                        dst = (sdst[c:c + 1, :] if sdst is not None
                               else sums16[4 * pn + c:4 * pn + c + 1, :])
                        eng.dma_start(
                            dst, ctx_sb[H:H + 1, c * 128:(c + 1) * 128],
                        )
                    ctx_keep.append((pn, ctx_sb))
                    # deferred normalization of j-1: one f16 reciprocal for
                    # all 16 rows after h0, broadcast batch after h2
                    if prev_norm[0] is not None:
                        p_sums, p_rh, p_ctx, pj = prev_norm[0]
                        if pn == 2:
                            for nn, c_sb in p_ctx:
                                _bc_out(nn, c_sb, p_rh, j=pj)
                            prev_norm[0] = None
                    if pn == 1 and mask_next_h[0] is not None:
                        # gate the 2MB prefetch behind h1 so it doesn't
                        # steal DMA bandwidth from the norm gathers
                        nc.vector.memset(mask_next_h[0][0:1, 0:1, 0:1], 0.0)
                        nc.sync.dma_start(mask_next_h[0][:], maskp[:, j + 1])
                        mask_next_h[0] = None
                    if final and pn == 2:
                        # early recip: heads 0-2 rows valid, head-3 rows are
                        # the 1.0 placeholder (rewritten before bc23 reads)
                        with nc.allow_low_precision(reason="softmax recip"):
                            nc.vector.reciprocal(recip16h[:], sums16[:])

                pends = []  # lagged exp groups, carried across heads; the
                # head's last group is held back past the next head's first
                # S so the exp stream never waits at a head boundary
                for n in range(HPC):
                    g_, par = divmod(n, 2)
                    KT_ = KTe if par == 0 else KTo
                    ps_ctx = ps_c.tile([HP1, FB], F32, tag="ctx", name="ctx")

                    for qi, w in enumerate(grps):
                        t0 = gt0[qi]
                        for piece in slots(j, n, qi):
                            if piece[0] == "k":
                                _emit_k_piece(nc, ps_util, kv, sbuf_t,
                                              piece[1], piece[2])
                            elif piece[0] == "v":
                                _emit_v_piece(nc, ps_util, kv, sbuf_t, piece[1])
                            else:
                                _emit_qt_piece(nc, ps_util, wq_sb, fromT_sb,
                                               QT, bias_sb, piece[1], piece[2])
                        ps_sq = ps_s.tile([128, w, FB], F32, tag="sq", name="sq")
                        for i in range(w):
                            nc.tensor.matmul(
                                ps_sq[:, i, :],
                                KT_[:, g_, (t0 + i) * 128:(t0 + i + 1) * 128],
                                QT[:, g_, j * FB:(j + 1) * FB],
                                start=True, stop=True,
                            )
                        ex = pexp.tile([128, w, FB], BF16, tag=exp_tag, name="exp")
                        nc.scalar.activation(
                            ex[:], ps_sq[:],
                            mybir.ActivationFunctionType.Exp,
                            scale=0.125,
                        )
                        nc.vector.tensor_mul(
                            ex[:], ex[:], mask_j[:, t0:t0 + w, :]
                        )
                        if qi < len(grps) - 1:
                            for p in pends:
                                _emit_pv(p)
                            pends = []
                        pends.append((n, ps_ctx, t0, w, ex))
                for p in pends:
                    _emit_pv(p)

                if not final:
                    with nc.allow_low_precision(reason="softmax recip"):
                        nc.vector.reciprocal(recip16h[:], sums16[:])
                    prev_norm[0] = (sums16, recip16h, list(ctx_keep), j)
                else:
                    # tail: heads 0-2 straight from the early recip (their
                    # rows were valid at h2), head 3 via its own [4,128]
                    # recip so the chain is just gathers+recip+bc
                    for nn, ctx_sb in ctx_keep[:3]:
                        _bc_out(nn, ctx_sb, recip16h)
                    with nc.allow_low_precision(reason="softmax recip -> f16 bcast"):
                        nc.vector.reciprocal(recip4[:], sums4[:])
                    for nn, ctx_sb in ctx_keep[3:]:
                        _bc_out(nn, ctx_sb, recip4)
            ps_u_cm.__exit__(None, None, None)
            ps_s_cm.__exit__(None, None, None)

    nc.compile()
    return nc


_compiled = None


def _get_compiled():
    global _compiled
    if _compiled is None:
        _compiled = _program()
    return _compiled


def make_in_maps(from_tensor, to_tensor, attention_mask, wq, bq, wk, bk, wv, bv):
    bf = ml_dtypes.bfloat16
    from_tensor = np.asarray(from_tensor, dtype=np.float32)
    to_tensor = np.asarray(to_tensor, dtype=np.float32)
    attention_mask = np.asarray(attention_mask)
    wq = np.asarray(wq, dtype=np.float32)
    wk = np.asarray(wk, dtype=np.float32)
    wv = np.asarray(wv, dtype=np.float32)
    bq = np.asarray(bq, dtype=np.float32)
    bk = np.asarray(bk, dtype=np.float32)
    bv = np.asarray(bv, dtype=np.float32)

    def pack_dt(x):
        # [S, D] activations -> [128, NJ, NK, FB] partition-major contiguous
        xT = x.T.astype(bf)                                   # [D, S]
        return np.ascontiguousarray(
            xT.reshape(NK, 128, NJ, FB).transpose(1, 2, 0, 3))

    def pack_mask(m):
        # [F, T] mask -> maskT [T, F] -> [128, NJ, NT, FB]
        mT = m.T.astype(bf)
        return np.ascontiguousarray(
            mT.reshape(NT, 128, NJ, FB).transpose(1, 2, 0, 3))

    def pack_w(w):
        # [D, HPC*H] -> [128, NK, HPC*H]
        return np.ascontiguousarray(
            w.astype(bf).reshape(NK, 128, HPC * H).transpose(1, 0, 2))

    fromT_b = [pack_dt(from_tensor[b]) for b in range(B)]
    toT_b = [pack_dt(to_tensor[b]) for b in range(B)]
    maskT_b = [pack_mask(attention_mask[b]) for b in range(B)]
    vones_arr = np.zeros((128, 128), dtype=bf)
    vones_arr[0, :] = 1.0
    ones_bc_arr = np.zeros((16, 16, 128), dtype=np.float16)
    for idx in range(16):
        ones_bc_arr[idx, idx, :] = 1.0

    in_maps = []
    for c in range(NCORES):
        b, hb = divmod(c, NCORES // B)
        hs = hb * HPC
        bq_dev = bq[hs:hs + HPC].reshape(NG, 128).T
        bk_dev = bk[hs:hs + HPC].reshape(NG, 128).T
        bv_pad = np.zeros((128, HPC * H), dtype=bf)
        bv_pad[0, :] = bv[hs:hs + HPC].reshape(HPC * H)
        in_maps.append(
            dict(
                fromTp=fromT_b[b],
                toTp=toT_b[b],
                maskp=maskT_b[b],
                wq=pack_w(wq[:, hs:hs + HPC, :].reshape(D, HPC * H)),
                wk=pack_w(wk[:, hs:hs + HPC, :].reshape(D, HPC * H)),
                wv=pack_w(wv[:, hs:hs + HPC, :].reshape(D, HPC * H)),
                bqk=np.ascontiguousarray(
                    np.concatenate([bq_dev, bk_dev], axis=1), dtype=np.float32
                ),
                bv_pad=bv_pad,
                vones=vones_arr,
                ones_bc=ones_bc_arr,
            )
        )
    return in_maps


def gather_output(results):
    out = np.empty((B, F, N, H), dtype=np.float32)
    for c in range(NCORES):
        b, hb = divmod(c, NCORES // B)
        hs = hb * HPC
        ctx = np.asarray(results[c]["out_ctx"], dtype=np.float32)  # [HPC, H, F]
        out[b, :, hs:hs + HPC, :] = ctx.transpose(2, 0, 1)
    return out


def run_sharded(inputs, **run_kwargs):
    """Run the SPMD kernel; returns (output, BassKernelResults)."""
    nc = _get_compiled()
    in_maps = make_in_maps(**inputs)
    res = run_bass_kernel_spmd(nc, in_maps, list(range(NCORES)), **run_kwargs)
    return gather_output(res.results), res


def kernel(**inputs):
    out, _ = run_sharded(inputs)
    return out
